# revision 9
# baseline (speedup 1.0000x reference)
"""Fused attention kernel for TRN2, data-parallel over 8 NeuronCores.

Problem: LN -> qk/v projections -> softplus-polar embedding -> attention
-> output projection.  B=8 batch elements are sharded one-per-core; each
core runs an identical single-core program (no collectives).

v2 design notes (vs the 390-466us baseline):
  The kernel is ACT-bound: the softmax exp stream (128 x [128,1024] from
  PSUM) plus the softplus Exp/Ln pairs are ~216us of ScalarE time that
  nothing else can absorb (only ACT has exp tables; GPSIMD has no PSUM
  port).  Everything else is arranged to hide under that stream:
  - ONE activation table set for the whole kernel
    (natural_log_exp_and_others: Exp, Ln, Identity, Copy).  The
    compiler's table pass naively picks exp_and_others/natural_log and
    reloaded tables 18x (~28us + stalls); we patch its table-membership
    view so every activation resolves to the unified set.  LN's rsqrt is
    computed as exp(-0.5*ln(var+eps)) to stay inside the set.
  - The per-pair DVE RECIPROCAL of the softmax denominators cost 8us
    each (iterative divide, FD-bound) = 63us total.  Now: L rows for
    pairs 0-4 are collected into DRAM and inverted in ONE batched
    reciprocal mid-loop; pairs 5-7 use exp(-ln(L)) on ACT in the tail
    where ACT is idle.  1/L is broadcast from DRAM rows (bf16).
  - Unnormalized O^T goes straight into ot_sb (bf16); the 1/L multiply
    is done in place, so no po_sb staging pool holds 16 heads alive.
  - ACTIVATE COPY drains moved off the hot ACT stream (xnT/out drains
    alternate DVE/ACT only in the ACT-idle head/tail phases).
  - LN normalize is one ACT Identity op per chunk with per-partition
    scale=rsig / bias=-mu*rsig APs; the transpose runs in bf16 (4x less
    PE than the old f32 identity-matmul transpose).
  - v-projection chunks 4-7 are emitted in the pre-loop region as PE
    filler so the PE never idles >3.4us (HAM re-throttle to 1.2GHz was
    costing ~2x on matmul rate in the baseline).
  - Attention core unchanged: q/k produced transposed with w_qk tiles
    stationary, S^T = k2.T @ q2 per 128-token m-tile, exp on ACT with
    fused DH^-0.5 scale, O_un^T = V'.T @ E^T with a ones-column carrying
    the denominator L onto psum row 64/65 (head pairs share one
    reciprocal row pair).
"""

import os

import ml_dtypes
import numpy as np

import concourse.bass as bass
import concourse.tile as tile
from concourse import bacc, mybir
from concourse.bass_utils import run_bass_kernel_spmd
from concourse.masks import make_identity

F32 = mybir.dt.float32
BF16 = mybir.dt.bfloat16
AF = mybir.ActivationFunctionType
ALU = mybir.AluOpType

B, N, D, H, DH = 8, 1024, 1024, 16, 64
NC_, DT_, EC_Q, MC_ = 8, 8, 8, 8  # n-chunks, d-tiles, q e-chunks, m-tiles
SCALE = DH ** -0.5

# ---------------------------------------------------------------------
# Keep every activation in ONE table set.  The insertion pass picks the
# first set containing each function (exp_and_others for Exp,
# natural_log for Ln) and so reloads tables on every Exp<->Ln
# alternation.  Restrict its membership view so Exp/Ln only appear in
# natural_log_exp_and_others (which genuinely contains both, plus
# Identity/Copy) -- the emitted BIR is valid for the real hardware
# tables, just with a single load.
import concourse.bacc as _bacc_mod
from concourse.hw_specs import get_activation_tables as _orig_gat


def _gat_unified(arch):
    tabs = dict(_orig_gat(arch))
    strip = tabs["natural_log_exp_and_others"]
    return {
        name: (funcs if name == "natural_log_exp_and_others" else funcs - strip)
        for name, funcs in tabs.items()
    }


_bacc_mod.get_activation_tables = _gat_unified


def _emit(tc):
    nc = tc.nc

    x_d = nc.dram_tensor("x", [N, D], F32, kind="ExternalInput").ap()
    wqk_d = nc.dram_tensor("wqk", [D, 2 * H * DH], BF16, kind="ExternalInput").ap()
    wv_d = nc.dram_tensor("wv", [D, H * DH], BF16, kind="ExternalInput").ap()
    wout_d = nc.dram_tensor("wout", [H * DH, D], BF16, kind="ExternalInput").ap()
    csq_d = nc.dram_tensor("csq", [128, N], BF16, kind="ExternalInput").ap()
    csk_d = nc.dram_tensor("csk", [128, N], BF16, kind="ExternalInput").ap()
    qbias_d = nc.dram_tensor("qbias", [128, 16], F32, kind="ExternalInput").ap()
    vbias_d = nc.dram_tensor("vbias", [1, H * DH], F32, kind="ExternalInput").ap()
    bout_d = nc.dram_tensor("bout", [1, D], BF16, kind="ExternalInput").ap()
    out_d = nc.dram_tensor("out", [N, D], F32, kind="ExternalOutput").ap()

    def bcast(ap_1xN, parts=128):
        return bass.AP(
            tensor=ap_1xN.tensor, offset=ap_1xN.offset, ap=[[0, parts]] + ap_1xN.ap[1:]
        )

    with (
        tc.tile_pool(name="const", bufs=1) as const,
        tc.tile_pool(name="xin", bufs=2) as xin,
        tc.tile_pool(name="ln", bufs=4) as ln,
        tc.tile_pool(name="xnbfp", bufs=2) as xnbfp,
        tc.tile_pool(name="spp", bufs=3) as spp,
        tc.tile_pool(name="q2p", bufs=4) as q2p,
        tc.tile_pool(name="k2p", bufs=4) as k2p,
        tc.tile_pool(name="etp", bufs=9) as etp,
        tc.tile_pool(name="otmpp", bufs=2) as otmpp,
        tc.tile_pool(name="llp", bufs=2) as llp,
        tc.tile_pool(name="lbcp", bufs=2) as lbcp,
        tc.tile_pool(name="drsp", bufs=1, space="DRAM") as drsp,
        tc.tile_pool(name="outp", bufs=1) as outp,
        tc.tile_pool(name="psA", bufs=2, space="PSUM") as psA,
        tc.tile_pool(name="psOun", bufs=2, space="PSUM") as psOun,
    ):
        # ---- resident constants -------------------------------------
        # identity first: the first transposes need it, and gpsimd compute
        # must not queue behind const-DMA issues
        ident = const.tile([128, 128], BF16, tag="ident")
        make_identity(nc, ident[:])
        # Only gpsimd/SP/ACT can issue DMAs; HBM aggregate ~358GB/s but a
        # single queue sustains much less, so the big loads are spread:
        # sync carries x (phase A), scalar carries wqk as per-t DMAs (the
        # qk accumulation consumes tiles in t order, so MMs start before
        # the whole 4MB lands), gpsimd carries smalls -> wv -> wout.
        qbias_sb = const.tile([128, 16], F32, tag="qbias")
        nc.gpsimd.dma_start(out=qbias_sb[:], in_=qbias_d)
        csq_sb = const.tile([128, N], BF16, tag="csq")
        nc.gpsimd.dma_start(out=csq_sb[:], in_=csq_d)
        csk_sb = const.tile([128, N], BF16, tag="csk")
        nc.gpsimd.dma_start(out=csk_sb[:], in_=csk_d)
        vb_sb = const.tile([128, 1024], F32, tag="vb")
        nc.gpsimd.dma_start(out=vb_sb[:], in_=bcast(vbias_d))
        bout_sb = const.tile([1, 1024], BF16, tag="bout")
        nc.gpsimd.dma_start(out=bout_sb[:], in_=bout_d)
        wqk_sb = const.tile([128, DT_, 2048], BF16, tag="wqk")
        wqk_r = wqk_d.rearrange("(t p) e -> p t e", p=128)
        for t in range(DT_):
            nc.scalar.dma_start(out=wqk_sb[:, t, :], in_=wqk_r[:, t, :])
        wv_sb = const.tile([128, DT_, 1024], BF16, tag="wv")
        nc.gpsimd.dma_start(out=wv_sb[:], in_=wv_d.rearrange("(t p) e -> p t e", p=128))
        # wout only needed at the tail
        wout_sb = const.tile([128, DT_, 1024], BF16, tag="wout")
        nc.gpsimd.dma_start(
            out=wout_sb[:], in_=wout_d.rearrange("(t p) e -> p t e", p=128)
        )
        ones_sb = const.tile([1, 128], BF16, tag="ones")
        nc.vector.memset(ones_sb[:], 1.0)
        eps_sb = const.tile([128, 1], F32, tag="eps")
        nc.vector.memset(eps_sb[:], 1e-5)

        xnT = const.tile([128, DT_, N], BF16, tag="xnT")
        vp = const.tile([128, MC_, H * 66], BF16, tag="vp")
        nc.vector.memset(vp[:], 1.0)
        # odd heads put their ones-column at position 65 (L lands on psum
        # row 65, a different partition than the even head's row 64, so a
        # pair shares one [2,1024] L-row block); position 64 must be 0.
        nc.vector.memset(
            vp.rearrange("p m (hp two w) -> p m hp two w", two=2, w=66)[
                :, :, :, 1, 64:65
            ],
            0.0,
        )
        ot_sb = const.tile([128, DT_, N], BF16, tag="otsb")
        # L staging: collected (raw) denominators and their inverses
        lA = const.tile([12, N], F32, tag="lA")
        linvA = const.tile([12, N], BF16, tag="linvA")
        lcoll_t = drsp.tile([12, N], F32, tag="lcoll")
        linv_d = drsp.tile([16, N], BF16, tag="linvd")

        # ---- Phase A: layernorm + PE transpose (bf16) ---------------
        def vproj(c):
            # v projection for n-chunk c: needs only chunk c of xnT
            psv = psA.tile([128, N], F32, tag="ps")
            for t in range(DT_):
                for hlf in range(2):
                    nc.tensor.matmul(
                        psv[:, hlf * 512 : (hlf + 1) * 512],
                        lhsT=xnT[:, t, c * 128 : (c + 1) * 128],
                        rhs=wv_sb[:, t, hlf * 512 : (hlf + 1) * 512],
                        start=(t == 0),
                        stop=(t == DT_ - 1),
                    )
            vpr = vp[:, c, :].rearrange("p (h w) -> p h w", w=66)
            nc.vector.tensor_add(
                out=vpr[:, :, 0:64],
                in0=psv.rearrange("p (h w) -> p h w", w=64),
                in1=vb_sb.rearrange("p (h w) -> p h w", w=64),
            )

        for c in range(NC_):
            x_t = xin.tile([128, D], F32, tag="x")
            nc.sync.dma_start(out=x_t[:], in_=x_d[c * 128 : (c + 1) * 128, :])
            st = ln.tile([128, 2, 6], F32, tag="st")
            for s in range(2):
                nc.vector.bn_stats(out=st[:, s, :], in_=x_t[:, s * 512 : (s + 1) * 512])
            mv = ln.tile([128, 2], F32, tag="mv")
            nc.vector.bn_aggr(out=mv[:], in_=st[:])
            # rsig = 1/sqrt(var+eps) = exp(-0.5*ln(var+eps)): stays in the
            # natural_log_exp table set (no Sqrt load, no DVE reciprocal)
            rsig = ln.tile([128, 1], F32, tag="rsig")
            nc.scalar.activation(rsig[:], mv[:, 1:2], AF.Ln, bias=eps_sb[:])
            nc.scalar.activation(rsig[:], rsig[:], AF.Exp, scale=-0.5)
            nmr = ln.tile([128, 1], F32, tag="nmr")
            nc.vector.tensor_scalar(
                out=nmr[:],
                in0=mv[:, 0:1],
                scalar1=rsig[:],
                scalar2=-1.0,
                op0=ALU.mult,
                op1=ALU.mult,
            )
            # xn = (x - mu) * rsig as one ACT pass: Identity(x*rsig + nmr)
            xnbf = xnbfp.tile([128, D], BF16, tag="xnbf")
            nc.scalar.activation(
                xnbf[:], x_t[:], AF.Identity, bias=nmr[:], scale=rsig[:]
            )
            pst = psA.tile([128, N], F32, tag="ps")
            for t in range(DT_):
                nc.tensor.matmul(
                    pst[:, t * 128 : (t + 1) * 128],
                    lhsT=xnbf[:, t * 128 : (t + 1) * 128],
                    rhs=ident[:],
                    start=True,
                    stop=True,
                )
            xdst = xnT[:, :, c * 128 : (c + 1) * 128]
            xsrc = pst.rearrange("p (t n) -> p t n", n=128)
            if c % 2 == 0:
                nc.scalar.copy(out=xdst, in_=xsrc)
            else:
                nc.vector.tensor_copy(out=xdst, in_=xsrc)

        # ---- helpers ------------------------------------------------
        def qk_compute(j):
            psqk = []
            for is_q in (True, False):
                ecol = j * 128 if is_q else 1024 + j * 128
                ps = psA.tile([128, N], F32, tag="ps")
                for t in range(DT_):
                    for hlf in range(2):
                        nc.tensor.matmul(
                            ps[:, hlf * 512 : (hlf + 1) * 512],
                            lhsT=wqk_sb[:, t, ecol : ecol + 128],
                            rhs=xnT[:, t, hlf * 512 : (hlf + 1) * 512],
                            start=(t == 0),
                            stop=(t == DT_ - 1),
                        )
                psqk.append(ps)
            for is_q, ps in zip((True, False), psqk):
                bcol = j if is_q else 8 + j
                nc.scalar.activation(
                    ps[:], ps[:], AF.Exp, bias=qbias_sb[:, bcol : bcol + 1]
                )
            sps = []
            for ps in psqk:
                sp = spp.tile([128, N], BF16, tag="sp")
                nc.scalar.activation(sp[:], ps[:], AF.Ln, bias=1.0)
                sps.append(sp)
            out = []
            for is_q, sp in zip((True, False), sps):
                pool = q2p if is_q else k2p
                cs = csq_sb if is_q else csk_sb
                tiles = []
                dq = nc.sync if is_q else nc.gpsimd
                for hh in range(2):
                    dup = pool.tile([128, N], BF16, tag="d")
                    dq.dma_start(
                        out=dup[0:64, :], in_=sp[hh * 64 : hh * 64 + 64, :]
                    )
                    dq.dma_start(
                        out=dup[64:128, :], in_=sp[hh * 64 : hh * 64 + 64, :]
                    )
                    nc.vector.tensor_mul(out=dup[:], in0=dup[:], in1=cs[:])
                    tiles.append(dup)
                out.append(tiles)
            return out

        et_tiles = {}

        def dots(h, q2, k2):
            ets = []
            for i in range(MC_):
                ps = psA.tile([128, N], F32, tag="ps")
                for hlf in range(2):
                    nc.tensor.matmul(
                        ps[:, hlf * 512 : (hlf + 1) * 512],
                        lhsT=k2[:, i * 128 : (i + 1) * 128],
                        rhs=q2[:, hlf * 512 : (hlf + 1) * 512],
                        start=True,
                        stop=True,
                    )
                et = etp.tile([128, N], BF16, tag="et")
                nc.scalar.activation(et[:], ps[:], AF.Exp, scale=SCALE)
                ets.append(et)
            et_tiles[h] = ets

        pair_po = {}
        pair_ll = {}

        def stage2(h):
            ets = et_tiles.pop(h)
            even = h % 2 == 0
            hp = h // 2
            ncols = 65 if even else 66  # odd: [v(64) | 0 | 1], L on row 65
            po = psOun.tile([128, N], F32, tag="oun")
            for i in range(MC_):
                for hlf in range(2):
                    nc.tensor.matmul(
                        po[0:ncols, hlf * 512 : (hlf + 1) * 512],
                        lhsT=vp[:, i, h * 66 : h * 66 + ncols],
                        rhs=ets[i][:, hlf * 512 : (hlf + 1) * 512],
                        start=(i == 0),
                        stop=(i == MC_ - 1),
                    )
            # drain unnormalized O straight into its ot_sb half (bf16);
            # the 1/L multiply happens in place later.
            if even:
                nc.vector.tensor_copy(out=ot_sb[0:64, hp, :], in_=po[0:64, :])
                pair_po[hp] = po  # keep psum ref for its L row (64)
            else:
                otmp = otmpp.tile([64, N], BF16, tag="otmp")
                nc.vector.tensor_copy(out=otmp[:], in_=po[0:64, :])
                nc.sync.dma_start(out=ot_sb[64:128, hp, :], in_=otmp[:])
                po_e = pair_po.pop(hp)
                ll = llp.tile([128, N], F32, tag="ll")
                # odd first: rows 64 (zero pad) + 65 (L_odd), then the
                # even head's L over the pad at row 64.  (DVE partition
                # slices must start at 0/32/64.)
                nc.vector.tensor_copy(out=ll[64:66, :], in_=po[64:66, :])
                nc.vector.tensor_copy(out=ll[64:65, :], in_=po_e[64:65, :])
                if hp <= 5:
                    nc.sync.dma_start(
                        out=lcoll_t[2 * hp : 2 * hp + 2, :], in_=ll[64:66, :]
                    )
                else:
                    pair_ll[hp] = ll

        def fin_pair(hp):
            # lbc rows 0:64 = 1/L_even, rows 64:128 = 1/L_odd
            lbc = lbcp.tile([128, N], BF16, tag="lbc")
            nc.sync.dma_start(
                out=lbc[0:64, :], in_=bcast(linv_d[2 * hp : 2 * hp + 1, :], 64)
            )
            nc.sync.dma_start(
                out=lbc[64:128, :], in_=bcast(linv_d[2 * hp + 1 : 2 * hp + 2, :], 64)
            )
            nc.vector.tensor_mul(
                out=ot_sb[0:64, hp, :], in0=ot_sb[0:64, hp, :], in1=lbc[0:64, :]
            )
            nc.vector.tensor_mul(
                out=ot_sb[64:128, hp, :], in0=ot_sb[64:128, hp, :], in1=lbc[64:128, :]
            )

        # ---- Phases B/C/D interleaved -------------------------------
        q0, k0 = qk_compute(0)
        vproj(0)
        vproj(1)
        vproj(2)
        vproj(3)
        nxt = qk_compute(1)
        vproj(4)
        vproj(5)
        vproj(6)
        vproj(7)
        dots(0, q0[0], k0[0])
        dots(1, q0[1], k0[1])

        # out-proj split: a chunk's t=0..6 contributions only need head
        # pairs 0-6 (normalized during the last loop iteration); t=7 +
        # bias close the psum group after the pair-7 finalize chain.
        op_ps = {}

        def outproj_partial(c):
            ps = psA.tile([128, N], F32, tag="ps")
            for t in range(DT_ - 1):
                for hlf in range(2):
                    nc.tensor.matmul(
                        ps[:, hlf * 512 : (hlf + 1) * 512],
                        lhsT=ot_sb[:, t, c * 128 : (c + 1) * 128],
                        rhs=wout_sb[:, t, hlf * 512 : (hlf + 1) * 512],
                        start=(t == 0),
                        stop=False,
                    )
            op_ps[c] = ps

        def outproj_final(c):
            ps = op_ps.pop(c)
            for hlf in range(2):
                nc.tensor.matmul(
                    ps[:, hlf * 512 : (hlf + 1) * 512],
                    lhsT=ot_sb[:, 7, c * 128 : (c + 1) * 128],
                    rhs=wout_sb[:, 7, hlf * 512 : (hlf + 1) * 512],
                    start=False,
                    stop=False,
                )
            for hlf in range(2):  # b_out via a K=1 ones-row matmul
                nc.tensor.matmul(
                    ps[:, hlf * 512 : (hlf + 1) * 512],
                    lhsT=ones_sb[:],
                    rhs=bout_sb[0:1, hlf * 512 : (hlf + 1) * 512],
                    start=False,
                    stop=True,
                )
            o_t = outp.tile([128, D], F32, tag="of")
            if c % 2 == 0:
                nc.scalar.copy(out=o_t[:], in_=ps[:])
            else:
                nc.vector.tensor_copy(out=o_t[:], in_=ps[:])
            dq = nc.sync if c % 2 == 0 else nc.gpsimd
            dq.dma_start(out=out_d[c * 128 : (c + 1) * 128, :], in_=o_t[:])

        for j in range(1, EC_Q):
            qj, kj = nxt
            # produce the NEXT step's pair first: dots below consume the
            # pair made last step, so the PE->ACT->DVE softplus/polar
            # chain has a full step of slack and never gates the PE.
            if j + 1 < EC_Q:
                nxt = qk_compute(j + 1)
            dots(2 * j, qj[0], kj[0])
            stage2(2 * j - 2)
            dots(2 * j + 1, qj[1], kj[1])
            stage2(2 * j - 1)
            if j == 6:
                # pairs 0-5 are collected; one batched reciprocal covers
                # their 12 L rows (DVE iterative divide is FD-bound, so
                # batching 12 rows costs the same as one pair did).
                nc.sync.dma_start(out=lA[0:12, :], in_=lcoll_t[0:12, :])
                with nc.allow_low_precision(reason="1/L broadcast in bf16 is ample"):
                    nc.vector.reciprocal(out=linvA[0:12, :], in_=lA[0:12, :])
                nc.sync.dma_start(out=linv_d[0:12, :], in_=linvA[0:12, :])
            if j == 7:
                for hp in range(6):
                    fin_pair(hp)
                # pair 6: in-place DVE reciprocal on the ll rows (runs
                # under the loop's ACT stream, no DRAM round trip)
                ll6 = pair_ll.pop(6)
                llinv6 = llp.tile([128, N], BF16, tag="llinv")
                with nc.allow_low_precision(reason="1/L broadcast in bf16 is ample"):
                    nc.vector.reciprocal(out=llinv6[64:66, :], in_=ll6[64:66, :])
                nc.sync.dma_start(out=linv_d[12:14, :], in_=llinv6[64:66, :])
                fin_pair(6)

        stage2(14)
        stage2(15)
        # pair 7: invert via exp(-ln(L)) on ACT (idle in the tail, and
        # ~2.6us beats an 8.5us FD-bound DVE reciprocal here); the first
        # two out-proj partials keep the PE warm under this chain.
        ll7 = pair_ll.pop(7)
        llinv7 = llp.tile([128, N], BF16, tag="llinv")
        nc.scalar.activation(ll7[64:66, :], ll7[64:66, :], AF.Ln)
        nc.scalar.activation(llinv7[64:66, :], ll7[64:66, :], AF.Exp, scale=-1.0)
        nc.sync.dma_start(out=linv_d[14:16, :], in_=llinv7[64:66, :])
        outproj_partial(0)
        outproj_partial(1)
        fin_pair(7)

        # ---- Phase F: output projection (finals + remaining partials)
        outproj_final(0)
        for c in range(2, NC_):
            outproj_partial(c)
            outproj_final(c - 1)
        outproj_final(NC_ - 1)


_NC_CACHE = {}


def _get_nc():
    if "nc" not in _NC_CACHE:
        nc = bacc.Bacc(
            "TRN2",
            target_bir_lowering=False,
            debug=False,
            enable_asserts=False,
            num_devices=8,
        )
        with tile.TileContext(nc) as tc:
            _emit(tc)
        nc.compile()
        _NC_CACHE["nc"] = nc
    return _NC_CACHE["nc"]


def _trace_ok():
    try:
        from antenv.axon_hooks import get_axon_ntff_profile_hook

        return get_axon_ntff_profile_hook() is not None
    except Exception:
        return False


def kernel(**inputs):
    bf = ml_dtypes.bfloat16
    x = np.ascontiguousarray(np.asarray(inputs["x"], dtype=np.float32))
    freqs = np.asarray(inputs["freqs"], dtype=np.float32)[0]
    fbias = np.asarray(inputs["bias"], dtype=np.float32)[0]
    g = np.asarray(inputs["ln_gamma"], dtype=np.float32)
    be = np.asarray(inputs["ln_beta"], dtype=np.float32)
    w_qk = np.asarray(inputs["w_qk"], dtype=np.float32)
    w_v = np.asarray(inputs["w_v"], dtype=np.float32)
    w_out = np.asarray(inputs["w_out"], dtype=np.float32)
    b_out = np.asarray(inputs["b_out"], dtype=np.float32)

    wqk_s = np.ascontiguousarray((w_qk * g[:, None]).astype(bf))
    wv_s = np.ascontiguousarray((w_v * g[:, None]).astype(bf))
    wout_b = np.ascontiguousarray(w_out.astype(bf))
    qb = be @ w_qk
    vb = (be @ w_v).astype(np.float32)[None, :]
    qbias = np.ascontiguousarray(qb.reshape(16, 128).T.astype(np.float32))
    csq = np.ascontiguousarray(
        np.concatenate([np.cos(freqs).T, np.sin(freqs).T], axis=0).astype(bf)
    )
    fb = freqs + fbias
    csk = np.ascontiguousarray(
        np.concatenate([np.cos(fb).T, np.sin(fb).T], axis=0).astype(bf)
    )
    bout = np.ascontiguousarray(b_out[None, :].astype(bf))

    shared = dict(
        wqk=wqk_s, wv=wv_s, wout=wout_b, csq=csq, csk=csk,
        qbias=qbias, vbias=vb, bout=bout,
    )
    in_maps = [dict(x=np.ascontiguousarray(x[i]), **shared) for i in range(B)]

    nc = _get_nc()
    want_trace = bool(int(os.environ.get("KERNEL_TRACE", "0")))
    res = run_bass_kernel_spmd(
        nc,
        in_maps,
        core_ids=list(range(B)),
        trace=want_trace and _trace_ok(),
    )
    out = np.stack([res.results[i]["out"] for i in range(B)], axis=0)
    if getattr(res, "exec_time_ns", None):
        kernel.last_exec_time_ns = res.exec_time_ns
    kernel.last_results = res
    return out


# revision 10
# speedup vs baseline: 1.0515x; 1.0515x over previous
"""Fused attention kernel for TRN2, data-parallel over 8 NeuronCores.

Problem: LN -> qk/v projections -> softplus-polar embedding -> attention
-> output projection.  B=8 batch elements are sharded one-per-core; each
core runs an identical single-core program (no collectives).

v2 design notes (vs the 390-466us baseline):
  The kernel is ACT-bound: the softmax exp stream (128 x [128,1024] from
  PSUM) plus the softplus Exp/Ln pairs are ~216us of ScalarE time that
  nothing else can absorb (only ACT has exp tables; GPSIMD has no PSUM
  port).  Everything else is arranged to hide under that stream:
  - ONE activation table set for the whole kernel
    (natural_log_exp_and_others: Exp, Ln, Identity, Copy).  The
    compiler's table pass naively picks exp_and_others/natural_log and
    reloaded tables 18x (~28us + stalls); we patch its table-membership
    view so every activation resolves to the unified set.  LN's rsqrt is
    computed as exp(-0.5*ln(var+eps)) to stay inside the set.
  - The per-pair DVE RECIPROCAL of the softmax denominators cost 8us
    each (iterative divide, FD-bound) = 63us total.  Now: L rows for
    pairs 0-4 are collected into DRAM and inverted in ONE batched
    reciprocal mid-loop; pairs 5-7 use exp(-ln(L)) on ACT in the tail
    where ACT is idle.  1/L is broadcast from DRAM rows (bf16).
  - Unnormalized O^T goes straight into ot_sb (bf16); the 1/L multiply
    is done in place, so no po_sb staging pool holds 16 heads alive.
  - ACTIVATE COPY drains moved off the hot ACT stream (xnT/out drains
    alternate DVE/ACT only in the ACT-idle head/tail phases).
  - LN normalize is one ACT Identity op per chunk with per-partition
    scale=rsig / bias=-mu*rsig APs; the transpose runs in bf16 (4x less
    PE than the old f32 identity-matmul transpose).
  - v-projection chunks 4-7 are emitted in the pre-loop region as PE
    filler so the PE never idles >3.4us (HAM re-throttle to 1.2GHz was
    costing ~2x on matmul rate in the baseline).
  - Attention core unchanged: q/k produced transposed with w_qk tiles
    stationary, S^T = k2.T @ q2 per 128-token m-tile, exp on ACT with
    fused DH^-0.5 scale, O_un^T = V'.T @ E^T with a ones-column carrying
    the denominator L onto psum row 64/65 (head pairs share one
    reciprocal row pair).
"""

import os

import ml_dtypes
import numpy as np

import concourse.bass as bass
import concourse.tile as tile
from concourse import bacc, mybir
from concourse.bass_utils import run_bass_kernel_spmd
from concourse.masks import make_identity

F32 = mybir.dt.float32
BF16 = mybir.dt.bfloat16
AF = mybir.ActivationFunctionType
ALU = mybir.AluOpType

B, N, D, H, DH = 8, 1024, 1024, 16, 64
NC_, DT_, EC_Q, MC_ = 8, 8, 8, 8  # n-chunks, d-tiles, q e-chunks, m-tiles
SCALE = DH ** -0.5

# ---------------------------------------------------------------------
# Keep every activation in ONE table set.  The insertion pass picks the
# first set containing each function (exp_and_others for Exp,
# natural_log for Ln) and so reloads tables on every Exp<->Ln
# alternation.  Restrict its membership view so Exp/Ln only appear in
# natural_log_exp_and_others (which genuinely contains both, plus
# Identity/Copy) -- the emitted BIR is valid for the real hardware
# tables, just with a single load.
import concourse.bacc as _bacc_mod
from concourse.hw_specs import get_activation_tables as _orig_gat


def _gat_unified(arch):
    tabs = dict(_orig_gat(arch))
    strip = tabs["natural_log_exp_and_others"]
    return {
        name: (funcs if name == "natural_log_exp_and_others" else funcs - strip)
        for name, funcs in tabs.items()
    }


_bacc_mod.get_activation_tables = _gat_unified


def _emit(tc):
    nc = tc.nc

    x_d = nc.dram_tensor("x", [N, D], F32, kind="ExternalInput").ap()
    wqk_d = nc.dram_tensor("wqk", [128, 16 * 1024], BF16, kind="ExternalInput").ap()
    wv_d = nc.dram_tensor("wv", [D, H * DH], BF16, kind="ExternalInput").ap()
    wout_d = nc.dram_tensor("wout", [H * DH, D], BF16, kind="ExternalInput").ap()
    csq_d = nc.dram_tensor("csq", [128, N], BF16, kind="ExternalInput").ap()
    csk_d = nc.dram_tensor("csk", [128, N], BF16, kind="ExternalInput").ap()
    qbias_d = nc.dram_tensor("qbias", [128, 16], F32, kind="ExternalInput").ap()
    vbias_d = nc.dram_tensor("vbias", [1, H * DH], F32, kind="ExternalInput").ap()
    bout_d = nc.dram_tensor("bout", [1, D], BF16, kind="ExternalInput").ap()
    out_d = nc.dram_tensor("out", [N, D], F32, kind="ExternalOutput").ap()

    def bcast(ap_1xN, parts=128):
        return bass.AP(
            tensor=ap_1xN.tensor, offset=ap_1xN.offset, ap=[[0, parts]] + ap_1xN.ap[1:]
        )

    with (
        tc.tile_pool(name="const", bufs=1) as const,
        tc.tile_pool(name="xin", bufs=2) as xin,
        tc.tile_pool(name="ln", bufs=4) as ln,
        tc.tile_pool(name="xnbfp", bufs=2) as xnbfp,
        tc.tile_pool(name="spp", bufs=3) as spp,
        tc.tile_pool(name="q2p", bufs=3) as q2p,
        tc.tile_pool(name="k2p", bufs=3) as k2p,
        tc.tile_pool(name="etp", bufs=9) as etp,
        tc.tile_pool(name="otmpp", bufs=2) as otmpp,
        tc.tile_pool(name="llp", bufs=2) as llp,
        tc.tile_pool(name="lbcp", bufs=2) as lbcp,
        tc.tile_pool(name="drsp", bufs=1, space="DRAM") as drsp,
        tc.tile_pool(name="outp", bufs=2) as outp,
        tc.tile_pool(name="psA", bufs=2, space="PSUM") as psA,
        tc.tile_pool(name="psOun", bufs=2, space="PSUM") as psOun,
    ):
        # ---- resident constants -------------------------------------
        # identity first: the first transposes need it, and gpsimd compute
        # must not queue behind const-DMA issues
        ident = const.tile([128, 128], BF16, tag="ident")
        make_identity(nc, ident[:])
        # Only gpsimd/SP/ACT can issue DMAs; HBM aggregate ~358GB/s but a
        # single queue sustains much less, so the big loads are spread:
        # sync carries x (phase A), scalar carries wqk as per-t DMAs (the
        # qk accumulation consumes tiles in t order, so MMs start before
        # the whole 4MB lands), gpsimd carries smalls -> wv -> wout.
        qbias_sb = const.tile([128, 16], F32, tag="qbias")
        nc.gpsimd.dma_start(out=qbias_sb[:], in_=qbias_d)
        csq_sb = const.tile([128, N], BF16, tag="csq")
        nc.gpsimd.dma_start(out=csq_sb[:], in_=csq_d)
        csk_sb = const.tile([128, N], BF16, tag="csk")
        nc.gpsimd.dma_start(out=csk_sb[:], in_=csk_d)
        # wqk pre-shuffled on host to [p, jj, t*128+e]; chunks jj=0 and
        # jj=8 land first so the progressive j=0 projection can start
        # during phase A.  4 DMAs on the scalar queue (cheap issues).
        wqk_sb = const.tile([128, 16, 1024], BF16, tag="wqk")
        wqk_r = wqk_d.rearrange("p (j w) -> p j w", w=1024)
        nc.scalar.dma_start(out=wqk_sb[:, 0:1, :], in_=wqk_r[:, 0:1, :])
        nc.scalar.dma_start(out=wqk_sb[:, 8:9, :], in_=wqk_r[:, 8:9, :])
        nc.scalar.dma_start(out=wqk_sb[:, 1:8, :], in_=wqk_r[:, 1:8, :])
        nc.scalar.dma_start(out=wqk_sb[:, 9:16, :], in_=wqk_r[:, 9:16, :])
        wv_sb = const.tile([128, DT_, 1024], BF16, tag="wv")
        nc.gpsimd.dma_start(out=wv_sb[:], in_=wv_d.rearrange("(t p) e -> p t e", p=128))
        vb_sb = const.tile([128, 1024], F32, tag="vb")
        nc.gpsimd.dma_start(out=vb_sb[:], in_=bcast(vbias_d))
        bout_sb = const.tile([1, 1024], BF16, tag="bout")
        nc.gpsimd.dma_start(out=bout_sb[:], in_=bout_d)
        # wout only needed at the tail
        wout_sb = const.tile([128, DT_, 1024], BF16, tag="wout")
        nc.gpsimd.dma_start(
            out=wout_sb[:], in_=wout_d.rearrange("(t p) e -> p t e", p=128)
        )
        ones_sb = const.tile([1, 128], BF16, tag="ones")
        nc.vector.memset(ones_sb[:], 1.0)
        eps_sb = const.tile([128, 1], F32, tag="eps")
        nc.vector.memset(eps_sb[:], 1e-5)

        xnT = const.tile([128, DT_, N], BF16, tag="xnT")
        vp = const.tile([128, MC_, H * 66], BF16, tag="vp")
        nc.vector.memset(vp[:], 1.0)
        # odd heads put their ones-column at position 65 (L lands on psum
        # row 65, a different partition than the even head's row 64, so a
        # pair shares one [2,1024] L-row block); position 64 must be 0.
        nc.vector.memset(
            vp.rearrange("p m (hp two w) -> p m hp two w", two=2, w=66)[
                :, :, :, 1, 64:65
            ],
            0.0,
        )
        ot_sb = const.tile([128, DT_, N], BF16, tag="otsb")
        # L staging: collected (raw) denominators and their inverses
        lA = const.tile([12, N], F32, tag="lA")
        linvA = const.tile([12, N], BF16, tag="linvA")
        lcoll_t = drsp.tile([12, N], F32, tag="lcoll")
        linv_d = drsp.tile([16, N], BF16, tag="linvd")

        # ---- Phase A: layernorm + PE transpose (bf16) ---------------
        def vproj(c):
            # v projection for n-chunk c: needs only chunk c of xnT
            psv = psA.tile([128, N], F32, tag="ps")
            for t in range(DT_):
                for hlf in range(2):
                    nc.tensor.matmul(
                        psv[:, hlf * 512 : (hlf + 1) * 512],
                        lhsT=xnT[:, t, c * 128 : (c + 1) * 128],
                        rhs=wv_sb[:, t, hlf * 512 : (hlf + 1) * 512],
                        start=(t == 0),
                        stop=(t == DT_ - 1),
                    )
            vpr = vp[:, c, :].rearrange("p (h w) -> p h w", w=66)
            nc.vector.tensor_add(
                out=vpr[:, :, 0:64],
                in0=psv.rearrange("p (h w) -> p h w", w=64),
                in1=vb_sb.rearrange("p (h w) -> p h w", w=64),
            )

        # progressive j=0 q/k projection: chunk c of xnT enables the
        # n-slice-c accumulation groups; runs on the otherwise-idle
        # psOun banks and keeps the PE busy (HAM-warm) through phase A.
        psq0 = psOun.tile([128, N], F32, tag="oun")
        psk0 = psOun.tile([128, N], F32, tag="oun")

        def qk0_slice(c):
            for ps, jj in ((psq0, 0), (psk0, 8)):
                for t in range(DT_):
                    nc.tensor.matmul(
                        ps[:, c * 128 : (c + 1) * 128],
                        lhsT=wqk_sb[:, jj, t * 128 : (t + 1) * 128],
                        rhs=xnT[:, t, c * 128 : (c + 1) * 128],
                        start=(t == 0),
                        stop=(t == DT_ - 1),
                    )

        for c in range(NC_):
            x_t = xin.tile([128, D], F32, tag="x")
            nc.sync.dma_start(out=x_t[:], in_=x_d[c * 128 : (c + 1) * 128, :])
            st = ln.tile([128, 2, 6], F32, tag="st")
            for s in range(2):
                nc.vector.bn_stats(out=st[:, s, :], in_=x_t[:, s * 512 : (s + 1) * 512])
            mv = ln.tile([128, 2], F32, tag="mv")
            nc.vector.bn_aggr(out=mv[:], in_=st[:])
            # rsig = 1/sqrt(var+eps) = exp(-0.5*ln(var+eps)): stays in the
            # natural_log_exp table set (no Sqrt load, no DVE reciprocal)
            rsig = ln.tile([128, 1], F32, tag="rsig")
            nc.scalar.activation(rsig[:], mv[:, 1:2], AF.Ln, bias=eps_sb[:])
            nc.scalar.activation(rsig[:], rsig[:], AF.Exp, scale=-0.5)
            nmr = ln.tile([128, 1], F32, tag="nmr")
            nc.vector.tensor_scalar(
                out=nmr[:],
                in0=mv[:, 0:1],
                scalar1=rsig[:],
                scalar2=-1.0,
                op0=ALU.mult,
                op1=ALU.mult,
            )
            # xn = (x - mu) * rsig as one ACT pass: Identity(x*rsig + nmr)
            xnbf = xnbfp.tile([128, D], BF16, tag="xnbf")
            nc.scalar.activation(
                xnbf[:], x_t[:], AF.Identity, bias=nmr[:], scale=rsig[:]
            )
            pst = psA.tile([128, N], F32, tag="ps")
            for t in range(DT_):
                nc.tensor.matmul(
                    pst[:, t * 128 : (t + 1) * 128],
                    lhsT=xnbf[:, t * 128 : (t + 1) * 128],
                    rhs=ident[:],
                    start=True,
                    stop=True,
                )
            xdst = xnT[:, :, c * 128 : (c + 1) * 128]
            xsrc = pst.rearrange("p (t n) -> p t n", n=128)
            if c % 2 == 0:
                nc.scalar.copy(out=xdst, in_=xsrc)
            else:
                nc.vector.tensor_copy(out=xdst, in_=xsrc)
            qk0_slice(c)

        # ---- helpers ------------------------------------------------
        def qk_compute(j, psqk=None):
            if psqk is None:
                psqk = []
                for is_q in (True, False):
                    jj = j if is_q else 8 + j
                    ps = psA.tile([128, N], F32, tag="ps")
                    for t in range(DT_):
                        for hlf in range(2):
                            nc.tensor.matmul(
                                ps[:, hlf * 512 : (hlf + 1) * 512],
                                lhsT=wqk_sb[:, jj, t * 128 : (t + 1) * 128],
                                rhs=xnT[:, t, hlf * 512 : (hlf + 1) * 512],
                                start=(t == 0),
                                stop=(t == DT_ - 1),
                            )
                    psqk.append(ps)
            for is_q, ps in zip((True, False), psqk):
                bcol = j if is_q else 8 + j
                nc.scalar.activation(
                    ps[:], ps[:], AF.Exp, bias=qbias_sb[:, bcol : bcol + 1]
                )
            sps = []
            for ps in psqk:
                sp = spp.tile([128, N], BF16, tag="sp")
                nc.scalar.activation(sp[:], ps[:], AF.Ln, bias=1.0)
                sps.append(sp)
            out = []
            for is_q, sp in zip((True, False), sps):
                pool = q2p if is_q else k2p
                cs = csq_sb if is_q else csk_sb
                tiles = []
                dq = nc.sync
                for hh in range(2):
                    dup = pool.tile([128, N], BF16, tag="d")
                    dq.dma_start(
                        out=dup[0:64, :], in_=sp[hh * 64 : hh * 64 + 64, :]
                    )
                    dq.dma_start(
                        out=dup[64:128, :], in_=sp[hh * 64 : hh * 64 + 64, :]
                    )
                    nc.vector.tensor_mul(out=dup[:], in0=dup[:], in1=cs[:])
                    tiles.append(dup)
                out.append(tiles)
            return out

        et_tiles = {}

        def dots(h, q2, k2):
            ets = []
            for i in range(MC_):
                ps = psA.tile([128, N], F32, tag="ps")
                for hlf in range(2):
                    nc.tensor.matmul(
                        ps[:, hlf * 512 : (hlf + 1) * 512],
                        lhsT=k2[:, i * 128 : (i + 1) * 128],
                        rhs=q2[:, hlf * 512 : (hlf + 1) * 512],
                        start=True,
                        stop=True,
                    )
                et = etp.tile([128, N], BF16, tag="et")
                nc.scalar.activation(et[:], ps[:], AF.Exp, scale=SCALE)
                ets.append(et)
            et_tiles[h] = ets

        pair_po = {}
        pair_ll = {}

        def stage2(h):
            ets = et_tiles.pop(h)
            even = h % 2 == 0
            hp = h // 2
            ncols = 65 if even else 66  # odd: [v(64) | 0 | 1], L on row 65
            po = psOun.tile([128, N], F32, tag="oun")
            for i in range(MC_):
                for hlf in range(2):
                    nc.tensor.matmul(
                        po[0:ncols, hlf * 512 : (hlf + 1) * 512],
                        lhsT=vp[:, i, h * 66 : h * 66 + ncols],
                        rhs=ets[i][:, hlf * 512 : (hlf + 1) * 512],
                        start=(i == 0),
                        stop=(i == MC_ - 1),
                    )
            # drain unnormalized O straight into its ot_sb half (bf16);
            # the 1/L multiply happens in place later.
            if even:
                nc.vector.tensor_copy(out=ot_sb[0:64, hp, :], in_=po[0:64, :])
                pair_po[hp] = po  # keep psum ref for its L row (64)
            else:
                otmp = otmpp.tile([64, N], BF16, tag="otmp")
                nc.vector.tensor_copy(out=otmp[:], in_=po[0:64, :])
                nc.sync.dma_start(out=ot_sb[64:128, hp, :], in_=otmp[:])
                po_e = pair_po.pop(hp)
                ll = llp.tile([128, N], F32, tag="ll")
                # odd first: rows 64 (zero pad) + 65 (L_odd), then the
                # even head's L over the pad at row 64.  (DVE partition
                # slices must start at 0/32/64.)
                nc.vector.tensor_copy(out=ll[64:66, :], in_=po[64:66, :])
                nc.vector.tensor_copy(out=ll[64:65, :], in_=po_e[64:65, :])
                if hp <= 5:
                    nc.sync.dma_start(
                        out=lcoll_t[2 * hp : 2 * hp + 2, :], in_=ll[64:66, :]
                    )
                else:
                    pair_ll[hp] = ll

        def fin_pair(hp):
            # lbc rows 0:64 = 1/L_even, rows 64:128 = 1/L_odd
            lbc = lbcp.tile([128, N], BF16, tag="lbc")
            nc.sync.dma_start(
                out=lbc[0:64, :], in_=bcast(linv_d[2 * hp : 2 * hp + 1, :], 64)
            )
            nc.sync.dma_start(
                out=lbc[64:128, :], in_=bcast(linv_d[2 * hp + 1 : 2 * hp + 2, :], 64)
            )
            nc.vector.tensor_mul(
                out=ot_sb[0:64, hp, :], in0=ot_sb[0:64, hp, :], in1=lbc[0:64, :]
            )
            nc.vector.tensor_mul(
                out=ot_sb[64:128, hp, :], in0=ot_sb[64:128, hp, :], in1=lbc[64:128, :]
            )

        # ---- Phases B/C/D interleaved -------------------------------
        q0, k0 = qk_compute(0, psqk=[psq0, psk0])
        vproj(0)
        vproj(1)
        vproj(2)
        vproj(3)
        nxt = qk_compute(1)
        vproj(4)
        vproj(5)
        vproj(6)
        vproj(7)
        dots(0, q0[0], k0[0])
        dots(1, q0[1], k0[1])

        # out-proj split: a chunk's t=0..6 contributions only need head
        # pairs 0-6 (normalized during the last loop iteration); t=7 +
        # bias close the psum group after the pair-7 finalize chain.
        op_ps = {}

        def outproj_partial(c):
            ps = psA.tile([128, N], F32, tag="ps")
            for t in range(DT_ - 1):
                for hlf in range(2):
                    nc.tensor.matmul(
                        ps[:, hlf * 512 : (hlf + 1) * 512],
                        lhsT=ot_sb[:, t, c * 128 : (c + 1) * 128],
                        rhs=wout_sb[:, t, hlf * 512 : (hlf + 1) * 512],
                        start=(t == 0),
                        stop=False,
                    )
            op_ps[c] = ps

        def outproj_final(c):
            ps = op_ps.pop(c)
            for hlf in range(2):
                nc.tensor.matmul(
                    ps[:, hlf * 512 : (hlf + 1) * 512],
                    lhsT=ot_sb[:, 7, c * 128 : (c + 1) * 128],
                    rhs=wout_sb[:, 7, hlf * 512 : (hlf + 1) * 512],
                    start=False,
                    stop=False,
                )
            for hlf in range(2):  # b_out via a K=1 ones-row matmul
                nc.tensor.matmul(
                    ps[:, hlf * 512 : (hlf + 1) * 512],
                    lhsT=ones_sb[:],
                    rhs=bout_sb[0:1, hlf * 512 : (hlf + 1) * 512],
                    start=False,
                    stop=True,
                )
            o_t = outp.tile([128, D], F32, tag="of")
            if c % 2 == 0:
                nc.scalar.copy(out=o_t[:], in_=ps[:])
            else:
                nc.vector.tensor_copy(out=o_t[:], in_=ps[:])
            dq = nc.sync if c % 2 == 0 else nc.gpsimd
            dq.dma_start(out=out_d[c * 128 : (c + 1) * 128, :], in_=o_t[:])

        for j in range(1, EC_Q):
            qj, kj = nxt
            # produce the NEXT step's pair first: dots below consume the
            # pair made last step, so the PE->ACT->DVE softplus/polar
            # chain has a full step of slack and never gates the PE.
            if j + 1 < EC_Q:
                nxt = qk_compute(j + 1)
            dots(2 * j, qj[0], kj[0])
            stage2(2 * j - 2)
            dots(2 * j + 1, qj[1], kj[1])
            stage2(2 * j - 1)
            if j == 6:
                # pairs 0-5 are collected; one batched reciprocal covers
                # their 12 L rows (DVE iterative divide is FD-bound, so
                # batching 12 rows costs the same as one pair did).
                nc.sync.dma_start(out=lA[0:12, :], in_=lcoll_t[0:12, :])
                with nc.allow_low_precision(reason="1/L broadcast in bf16 is ample"):
                    nc.vector.reciprocal(out=linvA[0:12, :], in_=lA[0:12, :])
                nc.sync.dma_start(out=linv_d[0:12, :], in_=linvA[0:12, :])
            if j == 7:
                for hp in range(6):
                    fin_pair(hp)
                # pair 6: in-place DVE reciprocal on the ll rows (runs
                # under the loop's ACT stream, no DRAM round trip)
                ll6 = pair_ll.pop(6)
                llinv6 = llp.tile([128, N], BF16, tag="llinv")
                with nc.allow_low_precision(reason="1/L broadcast in bf16 is ample"):
                    nc.vector.reciprocal(out=llinv6[64:66, :], in_=ll6[64:66, :])
                nc.sync.dma_start(out=linv_d[12:14, :], in_=llinv6[64:66, :])
                fin_pair(6)

        stage2(14)
        stage2(15)
        # pair 7: invert via exp(-ln(L)) on ACT (idle in the tail, and
        # ~2.6us beats an 8.5us FD-bound DVE reciprocal here); the first
        # two out-proj partials keep the PE warm under this chain.
        ll7 = pair_ll.pop(7)
        llinv7 = llp.tile([128, N], BF16, tag="llinv")
        nc.scalar.activation(ll7[64:66, :], ll7[64:66, :], AF.Ln)
        nc.scalar.activation(llinv7[64:66, :], ll7[64:66, :], AF.Exp, scale=-1.0)
        nc.sync.dma_start(out=linv_d[14:16, :], in_=llinv7[64:66, :])
        outproj_partial(0)
        outproj_partial(1)
        fin_pair(7)

        # ---- Phase F: output projection (finals + remaining partials)
        outproj_final(0)
        for c in range(2, NC_):
            outproj_partial(c)
            outproj_final(c - 1)
        outproj_final(NC_ - 1)


_NC_CACHE = {}


def _get_nc():
    if "nc" not in _NC_CACHE:
        nc = bacc.Bacc(
            "TRN2",
            target_bir_lowering=False,
            debug=False,
            enable_asserts=False,
            num_devices=8,
        )
        with tile.TileContext(nc) as tc:
            _emit(tc)
        nc.compile()
        _NC_CACHE["nc"] = nc
    return _NC_CACHE["nc"]


def _trace_ok():
    try:
        from antenv.axon_hooks import get_axon_ntff_profile_hook

        return get_axon_ntff_profile_hook() is not None
    except Exception:
        return False


def kernel(**inputs):
    bf = ml_dtypes.bfloat16
    x = np.ascontiguousarray(np.asarray(inputs["x"], dtype=np.float32))
    freqs = np.asarray(inputs["freqs"], dtype=np.float32)[0]
    fbias = np.asarray(inputs["bias"], dtype=np.float32)[0]
    g = np.asarray(inputs["ln_gamma"], dtype=np.float32)
    be = np.asarray(inputs["ln_beta"], dtype=np.float32)
    w_qk = np.asarray(inputs["w_qk"], dtype=np.float32)
    w_v = np.asarray(inputs["w_v"], dtype=np.float32)
    w_out = np.asarray(inputs["w_out"], dtype=np.float32)
    b_out = np.asarray(inputs["b_out"], dtype=np.float32)

    wqk_g = w_qk * g[:, None]
    # pre-shuffle wqk so each 128-col e-chunk jj is partition-contiguous:
    # wqkp[p, jj, t*128+e] = wqk[t*128+p, jj*128+e].  DMA elements become
    # 2KB+ runs per partition instead of 256B strided slivers.
    wqkp = np.ascontiguousarray(
        wqk_g.reshape(8, 128, 16, 128).transpose(1, 2, 0, 3).reshape(128, 16 * 1024)
    ).astype(bf)
    wv_s = np.ascontiguousarray((w_v * g[:, None]).astype(bf))
    wout_b = np.ascontiguousarray(w_out.astype(bf))
    qb = be @ w_qk
    vb = (be @ w_v).astype(np.float32)[None, :]
    qbias = np.ascontiguousarray(qb.reshape(16, 128).T.astype(np.float32))
    csq = np.ascontiguousarray(
        np.concatenate([np.cos(freqs).T, np.sin(freqs).T], axis=0).astype(bf)
    )
    fb = freqs + fbias
    csk = np.ascontiguousarray(
        np.concatenate([np.cos(fb).T, np.sin(fb).T], axis=0).astype(bf)
    )
    bout = np.ascontiguousarray(b_out[None, :].astype(bf))

    shared = dict(
        wqk=wqkp, wv=wv_s, wout=wout_b, csq=csq, csk=csk,
        qbias=qbias, vbias=vb, bout=bout,
    )
    in_maps = [dict(x=np.ascontiguousarray(x[i]), **shared) for i in range(B)]

    nc = _get_nc()
    want_trace = bool(int(os.environ.get("KERNEL_TRACE", "0")))
    res = run_bass_kernel_spmd(
        nc,
        in_maps,
        core_ids=list(range(B)),
        trace=want_trace and _trace_ok(),
    )
    out = np.stack([res.results[i]["out"] for i in range(B)], axis=0)
    if getattr(res, "exec_time_ns", None):
        kernel.last_exec_time_ns = res.exec_time_ns
    kernel.last_results = res
    return out
